# revision 28
# baseline (speedup 1.0000x reference)
"""DWHT (buggy in-place Walsh-Hadamard channel transform + channel shuffle) on 8 trn2 cores.

The whole nn.Module is a fixed linear map on the channel axis:
    y[b, :, h, w] = T @ x[b, :, h, w]
with T a (512, 256) matrix of small integers (|T| <= 13, exact in fp16).

Structure exploited (variant "u", default):
  * T = T2 @ A with A the channel-pair-sum map (every row of T is constant
    on input pairs), so the matmul contracts only k=128.
  * T has just 377 DISTINCT rows (100 exact-duplicate groups, 9 zero rows).
    The device computes the 377 unique output channels once (padded to
    384 = 3 m-tiles of 128) and the host unshard expands them to the full
    512 channels with a pure gather: y[b, c] = y_dev[b, CH_MAP[c]].  That
    cuts PSUM->SBUF copies and output DMA from 32 to 24 tiles per core.
  * The pair-sum s = x[::2] + x[1::2] is computed mostly by DMA: the even
    channels are loaded normally and the odd channels are accumulated into
    the same SBUF tile with a gpsimd (SWDGE) accum_op=add DMA - zero
    vector-engine cost.  The first two samples instead use split loads +
    DVE/Pool chunk adds to shorten the pipeline lead-in (a DMA completion
    semaphore costs ~2.6us, so accum chains are too slow for early tiles).
  * PSUM can only be read by DVE and ACT (gpsimd is rejected by the BIR
    verifier; dma_start requires SBUF/DRAM), so the fp32->fp16 downcast
    copies are the binding resource: ~21us of copy work over two engines.
    Everything else (input DMA, output DMA, matmuls) is scheduled around
    keeping those two copy streams dense.
  * All queue/engine assignments (tables below) were hill-climbed against
    the CoreSim cost model.
Batch 64 is sharded 8-ways (data parallel, 8 samples/core).

Precision: x is rounded to fp16 on the host (T2 and the pair-sum stay exact
in fp16; PSUM accumulates fp32), output is written fp16 and widened to fp32
on the host: measured rel err ~3.7e-4 vs the fp32 reference, far inside the
2e-2 gate.
"""

import os
import sys

import numpy as np

for _p in ("/opt/trn_rl_repo", "/root/.axon_site/_ro/trn_rl_repo"):
    if os.path.isdir(_p) and _p not in sys.path:
        sys.path.append(_p)

B, C_IN, C_OUT, HH, WW = 64, 256, 512, 28, 28
S = HH * WW  # 784
N_CORES = 8
BS = B // N_CORES  # 8 samples per core
N_PASSES, GROUPS = 8, 8

VARIANT = os.environ.get("DWHT_VARIANT", "u")

# spatial split per PSUM bank (each chunk <= 512 fp32 = one bank)
N_CHUNKS = ((0, 392), (392, 392))


def _dwht_T() -> np.ndarray:
    """Build the (512, 256) transform matrix by running the reference
    butterfly (including its partial-update in-place semantics) on identity."""
    x = np.zeros((C_OUT, C_IN), np.float64)
    x[:C_IN] = np.eye(C_IN)
    half = C_OUT // 2
    for _ in range(N_PASSES):
        top = x[::2] + x[1::2]
        x = x.copy()
        x[:half] = top
        bottom = x[::2] - x[1::2]
        x[half:] = bottom
    # channel shuffle with groups=8
    x = x.reshape(GROUPS, C_OUT // GROUPS, C_IN).transpose(1, 0, 2).reshape(C_OUT, C_IN)
    return x


# ----------------------------------------------------------------- u variant
#
# T has only 377 distinct rows (100 exact-duplicate groups covering 235 rows,
# 9 of them all-zero).  The device computes the 377 unique channels once
# (padded to 384 = 3 m-tiles of 128) and the host unshard gathers/expands to
# the full 512 channels: y[b, c] = y_dev[b, CH_MAP[c]].  This cuts the
# PSUM->SBUF copy count 32->24 and the output DMA 32->24 tiles per core.

_U_PAD = 384  # 377 unique rows padded to 3 tiles of 128


def _unique_map():
    """(T2u_padded (384, 128), ch_map (512,)) with T[c] == T2u[ch_map[c]] @ A."""
    T = _dwht_T()
    first: dict[bytes, int] = {}
    uniq_rows = []
    ch_map = np.zeros(C_OUT, np.int32)
    for c in range(C_OUT):
        k = T[c].tobytes()
        if k not in first:
            first[k] = len(uniq_rows)
            uniq_rows.append(c)
        ch_map[c] = first[k]
    t2u = T[uniq_rows][:, ::2]  # (n_uniq, 128)
    assert t2u.shape[0] <= _U_PAD, t2u.shape
    t2u = np.pad(t2u, ((0, _U_PAD - t2u.shape[0]), (0, 0)))
    return t2u, ch_map


# Schedule config, tunable for hill-climbing against the CoreSim cost model.
# in_mode: per sample: "0" split-halves+chunk-adds, "v"/"g" xs-load + add on
#   DVE/Pool, "a" even-load + gpsimd accum-DMA (queue of the even load is
#   given in in_q).  in_q: queue of the xs / even load.
# order: sample processing order (by expected readiness).
# copy/outq: per (position-in-order, mtile).
_U_CFG = dict(
    in_mode="00gg" + "aaaa",
    in_q="--yy" + "ygyy",
    adds="gvggvgvg",  # chunk0/chunk1 add engine for '0' samples
    order=(0, 1, 2, 3, 4, 5, 6, 7),
    in_issue=(0, 1, 4, 2, 5, 3, 6, 7),  # input DMA issue order
    copy="svsvsvsvsvsvsvsvsvsvssvv",
    outq="ygygggygygygyyyyyyyggyygyg",
    lookahead=3,
    tail_split=1,  # how many trailing m-tiles get split into chunk halves
    tail_eng="ssvsvsvs",
    tail_q="gsgsygyg",
    warmup=0,
)
if os.environ.get("DWHT_U_CFG"):
    import json as _json

    _U_CFG.update(_json.loads(os.environ["DWHT_U_CFG"]))


def _build_u(reps=1, cfg=None):
    import concourse.mybir as mybir
    from concourse import bacc
    from concourse.tile import TileContext

    cfg = dict(_U_CFG, **(cfg or {}))
    f32 = mybir.dt.float32
    fp16 = mybir.dt.float16

    nc = bacc.Bacc(None, target_bir_lowering=False)
    x = nc.dram_tensor("x", (BS, C_IN, S), fp16, kind="ExternalInput")
    tt = nc.dram_tensor("tt", (C_IN // 2, _U_PAD), fp16, kind="ExternalInput")
    y = nc.dram_tensor("y", (BS, _U_PAD, S), fp16, kind="ExternalOutput")

    def q(ch):
        return {"y": nc.sync, "s": nc.scalar, "g": nc.gpsimd, "v": nc.vector}[ch]

    order = list(cfg["order"])
    assert sorted(order) == list(range(BS))
    in_mode, in_q = cfg["in_mode"], cfg["in_q"]
    copy_t, outq_t = cfg["copy"], cfg["outq"]
    adds_t = cfg["adds"] + "vg" * max(0, 8 - len(cfg["adds"]) // 2)
    copy_t = copy_t + "v" * max(0, 24 - len(copy_t))
    outq_t = outq_t + "yg" * max(0, 12 - len(outq_t) // 2)
    in_issue = list(cfg.get("in_issue") or order)
    assert sorted(in_issue) == list(range(BS))

    with TileContext(nc) as tc:
        with (
            tc.tile_pool(name="w", bufs=1) as wp,
            tc.tile_pool(name="io", bufs=3) as io,
            tc.tile_pool(name="ps", bufs=4, space="PSUM") as pp,
        ):
            tt2 = wp.tile([128, _U_PAD], fp16, tag="tt2")
            q(cfg.get("tt2_q", "y")).dma_start(out=tt2[:], in_=tt[:, :])

            warmup = int(cfg.get("warmup", 0))
            if warmup:
                # dummy matmuls on a zeroed tile: burn the PE p-state ramp
                # before real data arrives
                wz = wp.tile([128, 512], fp16, tag="wz")
                nc.gpsimd.memset(wz[:], 0)
                wps = pp.tile([128, 2, 512], f32, tag="ps")
                for _ in range(warmup):
                    nc.tensor.matmul(
                        wps[:, 0, :], wz[:, 0:128], wz[:], start=True, stop=True
                    )

            xss = {}  # sample -> xs tile (modes 0/v/g)
            sss = {}  # sample -> ss tile
            acc_pending = []  # accum-mode samples whose accum is not yet issued

            def issue_accum(s):
                ss, src = sss[s][0], sss[s][1]
                nc.gpsimd.dma_start(
                    out=ss[:], in_=src[:, 1, :], accum_op=mybir.AluOpType.add
                )
                sss[s] = (ss, None)

            for s in in_issue:
                src = x[s].rearrange("(p two) f -> p two f", two=2)
                mode = in_mode[s]
                if mode == "0":
                    # spatial split across SP and Pool; each chunk's add
                    # depends only on its own half.  swap0 flips which queue
                    # carries the first-needed chunk.
                    xs = io.tile([128, 2, S], fp16, tag="xs", bufs=4)
                    q0, q1 = ("g", "y") if cfg.get("swap0") else ("y", "g")
                    q(q0).dma_start(out=xs[:, :, 0:392], in_=src[:, :, 0:392])
                    q(q1).dma_start(out=xs[:, :, 392:S], in_=src[:, :, 392:S])
                    xss[s] = xs
                elif mode in "vg":
                    xs = io.tile([128, 2, S], fp16, tag="xs", bufs=4)
                    q(in_q[s]).dma_start(out=xs[:], in_=src)
                    xss[s] = xs
                else:  # accum: even load now; gpsimd odd-accum issued later
                    ss = io.tile([128, S], fp16, tag="ss", bufs=BS)
                    q(in_q[s]).dma_start(out=ss[:], in_=src[:, 0, :])
                    sss[s] = (ss, src)
            acc_pending = [s for s in order if in_mode[s] == "a"]

            # issue the first few accums up front; the rest interleave with
            # compute so the Pool queue alternates accums and output stores
            lookahead = int(cfg.get("lookahead", 2))
            for s in acc_pending[:lookahead]:
                issue_accum(s)
            acc_pending = acc_pending[lookahead:]

            for si, s in enumerate(order):
                if acc_pending:
                    issue_accum(acc_pending.pop(0))
                mode = in_mode[s]
                if mode == "0":
                    xs = xss.pop(s)
                    ss = io.tile([128, S], fp16, tag="ssc", bufs=4)
                    for ci, (n0, nsz) in enumerate(N_CHUNKS):
                        nsl = slice(n0, n0 + nsz)
                        eng = nc.vector if adds_t[2 * s + ci] == "v" else nc.gpsimd
                        with tc.high_priority():
                            eng.tensor_add(ss[:, nsl], xs[:, 0, nsl], xs[:, 1, nsl])
                elif mode in "vg":
                    xs = xss.pop(s)
                    ss = io.tile([128, S], fp16, tag="ssc", bufs=4)
                    eng = nc.vector if mode == "v" else nc.gpsimd
                    with tc.high_priority():
                        eng.tensor_add(ss[:], xs[:, 0], xs[:, 1])
                else:
                    ss = sss.pop(s)[0]

                n_tiles = BS * (_U_PAD // 128)
                for m in range(_U_PAD // 128):
                    gi = si * 3 + m
                    msl = slice(m * 128, (m + 1) * 128)
                    ps = pp.tile([128, 2, 512], f32, tag="ps")
                    for ci, (n0, nsz) in enumerate(N_CHUNKS):
                        nc.tensor.matmul(
                            ps[:, ci, 0:nsz],
                            tt2[:, msl],
                            ss[:, n0 : n0 + nsz],
                            start=True,
                            stop=True,
                        )
                    n_split = int(cfg["tail_split"])
                    if gi >= n_tiles - n_split:
                        # drain: chunk halves on both copy engines into
                        # separate tiles (same-tile writes would serialize),
                        # stores on both big queues
                        k = 2 * (gi - (n_tiles - n_split))
                        tail_eng = cfg.get("tail_eng", "vsvsvsvs")
                        tail_q = cfg.get("tail_q", "ygygygyg")
                        for ci in range(2):
                            ysh = io.tile([128, 392], fp16, tag="ysh", bufs=4)
                            srcp = ps[:, ci, 0:392]
                            if tail_eng[k + ci] == "v":
                                nc.vector.tensor_copy(ysh[:], srcp)
                            else:
                                nc.scalar.copy(ysh[:], srcp)
                            q(tail_q[k + ci]).dma_start(
                                out=y[s, msl, ci * 392 : (ci + 1) * 392],
                                in_=ysh[:],
                            )
                    else:
                        ysm = io.tile([128, S], fp16, tag="ysm", bufs=int(cfg.get("ysm_bufs", 8)))
                        ysv = ysm.rearrange("p (c n) -> p c n", c=2)
                        dst = ysv[:, :, 0:392]
                        srcp = ps[:, :, 0:392]
                        if copy_t[gi] == "v":
                            nc.vector.tensor_copy(dst, srcp)
                        else:
                            nc.scalar.copy(dst, srcp)
                        q(outq_t[gi]).dma_start(out=y[s, msl, :], in_=ysm[:])

    nc.compile()
    return nc


# ---------------------------------------------------------------- t2 variant

# engine assignment tables, tuned against the CoreSim cost model.
# copy engines per (sample, m-tile): v=vector(DVE) s=scalar(ACT) g=gpsimd(Pool)
# out-dma queues per (sample, m-tile): y=sync(SP) s=scalar(ACT) g=gpsimd(Pool)
# in-dma queue per sample.
# Balance rationale: PSUM->SBUF copies may only run on DVE and ACT (gpsimd
# has no PSUM access), so the SP and Pool queues carry nearly all DMA
# traffic and ACT's queue share goes to copies instead.
_COPY_ENG = os.environ.get(
    "DWHT_COPY", "vsvs" + "vsvs" + "vsvs" + "vsvs" + "vsvs" + "vsss" + "vsss" + "vsvs"
)
_OUT_Q = os.environ.get(
    "DWHT_OUTQ", "ygyg" + "ygyy" + "ygyg" + "ygyy" + "ygyg" + "ygyg" + "ygyg" + "ygys"
)
_IN_Q = os.environ.get("DWHT_INQ", "-ygygygg")


def _build_t2(reps=1):
    import concourse.mybir as mybir
    from concourse import bacc
    from concourse.tile import TileContext

    f32 = mybir.dt.float32
    fp16 = mybir.dt.float16

    nc = bacc.Bacc(None, target_bir_lowering=False)
    x = nc.dram_tensor("x", (BS, C_IN, S), fp16, kind="ExternalInput")
    tt = nc.dram_tensor("tt", (C_IN // 2, C_OUT), fp16, kind="ExternalInput")
    y = nc.dram_tensor("y", (BS, C_OUT, S), fp16, kind="ExternalOutput")

    def q(ch):
        return {"y": nc.sync, "s": nc.scalar, "g": nc.gpsimd, "v": nc.vector}[ch]

    with TileContext(nc) as tc:
        with (
            tc.tile_pool(name="w", bufs=1) as wp,
            tc.tile_pool(name="io", bufs=3) as io,
            tc.tile_pool(name="ps", bufs=4, space="PSUM") as pp,
        ):
            tt2 = wp.tile([128, C_OUT], fp16, tag="tt2")
            nc.sync.dma_start(out=tt2[:], in_=tt[:, :])

            sample_seq = [s for _ in range(reps) for s in range(BS)]
            n_seq = len(sample_seq)
            xss = {}
            # all input loads upfront: input traffic is the only DMA work
            # that exists before the first PSUM tile is ready, so it must
            # fill the queues' lead-in window
            for si in range(n_seq):
                s = sample_seq[si]
                xs = io.tile([128, 2, S], fp16, tag="xs", bufs=BS)
                src = x[s].rearrange("(p two) f -> p two f", two=2)
                if si == 0:
                    # split the first load across two queues so the fill
                    # critical path is one half-transfer, not a full one
                    nc.sync.dma_start(out=xs[:, :, 0:392], in_=src[:, :, 0:392])
                    nc.gpsimd.dma_start(out=xs[:, :, 392:S], in_=src[:, :, 392:S])
                else:
                    q(_IN_Q[s]).dma_start(out=xs[:], in_=src)
                xss[si] = xs

            for si, s in enumerate(sample_seq):
                xs = xss.pop(si)
                # pair-sum on DVE (fp16, packed, SBUF -> 2x mode); split for
                # the first sample so matmuls can start on the first half
                ss = io.tile([128, S], fp16, tag="ss", bufs=4)
                if si == 0:
                    for n0, nsz in N_CHUNKS:
                        nsl = slice(n0, n0 + nsz)
                        nc.vector.tensor_add(ss[:, nsl], xs[:, 0, nsl], xs[:, 1, nsl])
                elif s % 2 == 1:
                    with tc.high_priority():
                        nc.gpsimd.tensor_add(ss[:], xs[:, 0], xs[:, 1])
                else:
                    with tc.high_priority():
                        nc.vector.tensor_add(ss[:], xs[:, 0], xs[:, 1])

                last = si == n_seq - 1
                for m in range(C_OUT // 128):
                    msl = slice(m * 128, (m + 1) * 128)
                    ps = pp.tile([128, 2, 512], f32, tag="ps")
                    for ci, (n0, nsz) in enumerate(N_CHUNKS):
                        nc.tensor.matmul(
                            ps[:, ci, 0:nsz],
                            tt2[:, msl],
                            ss[:, n0 : n0 + nsz],
                            start=True,
                            stop=True,
                        )
                    ysm = io.tile([128, S], fp16, tag="ysm", bufs=8)
                    ysv = ysm.rearrange("p (c n) -> p c n", c=2)
                    if last:
                        # drain fast: per-chunk copies and stores spread over
                        # engines/queues so the tail is one half-tile long
                        cengs = ("v", "s") if m % 2 == 0 else ("s", "v")
                        qouts = ("y", "s", "g", "y", "s", "g", "y", "s")
                        for ci in range(2):
                            dst = ysv[:, ci, 0:392]
                            src = ps[:, ci, 0:392]
                            if cengs[ci] == "v":
                                nc.vector.tensor_copy(dst, src)
                            else:
                                nc.scalar.copy(dst, src)
                            q(qouts[m * 2 + ci]).dma_start(
                                out=y[s, msl, ci * 392 : (ci + 1) * 392],
                                in_=ysm[:, ci * 392 : (ci + 1) * 392],
                            )
                    else:
                        dst = ysv[:, :, 0:392]
                        src = ps[:, :, 0:392]
                        eng = _COPY_ENG[s * 4 + m]
                        if eng == "v":
                            nc.vector.tensor_copy(dst, src)
                        else:
                            nc.scalar.copy(dst, src)
                        q(_OUT_Q[s * 4 + m]).dma_start(out=y[s, msl, :], in_=ysm[:])

    nc.compile()
    return nc


# ------------------------------------------------------- fp16_lo (old) variant


def _build_fp16lo(reps=1):
    import concourse.mybir as mybir
    from concourse import bacc
    from concourse.tile import TileContext

    f32 = mybir.dt.float32
    bf16 = mybir.dt.bfloat16
    fp16 = mybir.dt.float16

    nc = bacc.Bacc(None, target_bir_lowering=False)
    x = nc.dram_tensor("x", (BS, C_IN, S), f32, kind="ExternalInput")
    tt = nc.dram_tensor("tt", (C_IN, C_OUT), bf16, kind="ExternalInput")
    y = nc.dram_tensor("y", (BS, C_OUT, S), f32, kind="ExternalOutput")

    with TileContext(nc) as tc:
        with (
            tc.tile_pool(name="w", bufs=1) as wp,
            tc.tile_pool(name="io", bufs=3) as io,
            tc.tile_pool(name="ps", bufs=8, space="PSUM") as pp,
        ):
            tts = []
            tth = []
            for k in range(2):
                t = wp.tile([128, C_OUT], bf16, tag=f"tt{k}")
                nc.sync.dma_start(out=t[:], in_=tt[k * 128 : (k + 1) * 128, :])
                tts.append(t)
                h = wp.tile([128, C_OUT], fp16, tag=f"tth{k}")
                nc.vector.tensor_copy(h[:], t[:])
                tth.append(h)

            sample_seq = [s for _ in range(reps) for s in range(BS)]
            for si, s in enumerate(sample_seq):
                last_sample = si == len(sample_seq) - 1
                first_sample = si == 0
                xsk = []
                for k in range(2):
                    xs = io.tile([128, S], f32, tag="xs", bufs=8)
                    if first_sample:
                        for n0, nsz in N_CHUNKS:
                            nc.scalar.dma_start(
                                out=xs[:, n0 : n0 + nsz],
                                in_=x[s, k * 128 : (k + 1) * 128, n0 : n0 + nsz],
                            )
                    else:
                        nc.scalar.dma_start(
                            out=xs[:], in_=x[s, k * 128 : (k + 1) * 128, :]
                        )
                    xsk.append(xs)

                xhk = [
                    xs.bitcast(bf16).rearrange("p (f two) -> p f two", two=2)[:, :, 1]
                    for xs in xsk
                ]
                xlk = []
                for k in range(2):
                    xl = io.tile([128, S], fp16, tag="xl", bufs=8)
                    if first_sample:
                        for n0, nsz in N_CHUNKS:
                            nsl = slice(n0, n0 + nsz)
                            nc.vector.tensor_sub(
                                xl[:, nsl], xsk[k][:, nsl], xhk[k][:, nsl]
                            )
                    else:
                        nc.vector.tensor_sub(xl[:], xsk[k][:], xhk[k])
                    xlk.append(xl)
                passes = [
                    (xlk[0][:], tth[0]),
                    (xhk[0], tts[0]),
                    (xlk[1][:], tth[1]),
                    (xhk[1], tts[1]),
                ]

                for m in range(C_OUT // 128):
                    msl = slice(m * 128, (m + 1) * 128)
                    ysm = io.tile([128, S], f32, tag="ysm", bufs=6)
                    for ni, (n0, nsz) in enumerate(N_CHUNKS):
                        nsl = slice(n0, n0 + nsz)
                        ps = pp.tile([128, nsz], f32, tag="ps")
                        for i, (src, w) in enumerate(passes):
                            nc.tensor.matmul(
                                ps[:],
                                w[:, msl],
                                src[:, nsl],
                                start=(i == 0),
                                stop=(i == len(passes) - 1),
                            )
                        dst = ysm[:, nsl]
                        if last_sample:
                            if ni == 0:
                                nc.vector.tensor_copy(dst, ps[:])
                                nc.sync.dma_start(out=y[s, msl, nsl], in_=dst)
                            else:
                                nc.scalar.copy(dst, ps[:])
                                nc.scalar.dma_start(out=y[s, msl, nsl], in_=dst)
                        elif (m * len(N_CHUNKS) + ni) % 2 == 0:
                            nc.vector.tensor_copy(dst, ps[:])
                        else:
                            nc.scalar.copy(dst, ps[:])
                    if not last_sample:
                        nc.sync.dma_start(out=y[s, msl, :], in_=ysm[:])

    nc.compile()
    return nc


_cache = {}


def _get_nc(variant=None, reps=1):
    variant = variant or VARIANT
    key = (variant, reps)
    if key not in _cache:
        if variant == "u":
            _cache[key] = _build_u(reps)
        elif variant == "t2":
            _cache[key] = _build_t2(reps)
        elif variant == "fp16_lo":
            _cache[key] = _build_fp16lo(reps)
        else:
            raise ValueError(variant)
    return _cache[key]


def _in_maps(x_np, variant):
    if variant == "u":
        t2u, _ = _unique_map()
        tt_np = np.ascontiguousarray(t2u.T).astype(np.float16)  # (128, 384)
        x16 = x_np.astype(np.float16)
        return [
            {"x": x16[i * BS : (i + 1) * BS], "tt": tt_np} for i in range(N_CORES)
        ]
    T = _dwht_T()
    if variant == "t2":
        tt_np = np.ascontiguousarray(T[:, ::2].T).astype(np.float16)  # (128, 512)
        x16 = x_np.astype(np.float16)
        return [
            {"x": x16[i * BS : (i + 1) * BS], "tt": tt_np} for i in range(N_CORES)
        ]
    import ml_dtypes

    ttT = np.ascontiguousarray(T.T)  # (256, 512), lhsT layout
    tt_np = ttT.astype(ml_dtypes.bfloat16)
    return [
        {"x": x_np[i * BS : (i + 1) * BS], "tt": tt_np} for i in range(N_CORES)
    ]


def _run(x_np, variant=None, trace=False, reps=1):
    from concourse.bass_utils import run_bass_kernel_spmd

    variant = variant or VARIANT
    nc = _get_nc(variant, reps)
    res = run_bass_kernel_spmd(
        nc, _in_maps(x_np, variant), list(range(N_CORES)), trace=trace
    )
    if variant == "u":
        _, ch_map = _unique_map()
        yd = np.stack([r["y"] for r in res.results])  # (8, BS, 384, S) fp16
        y = yd[:, :, ch_map, :].astype(np.float32)
    else:
        y = np.stack([r["y"] for r in res.results]).astype(np.float32)
    y = y.reshape(B, C_OUT, HH, WW)
    return y, res


def kernel(x: np.ndarray) -> np.ndarray:
    x_np = np.ascontiguousarray(np.asarray(x), dtype=np.float32).reshape(B, C_IN, S)
    y, _ = _run(x_np)
    return y

